# revision 30
# baseline (speedup 1.0000x reference)
"""DeltaNet Trainium2 kernel — 8-core SPMD, one (batch, head) pair per core.

Full inputs -> shard on host -> Bass/Tile kernel per core -> host unshard.

v3 design:
  * bf16 inputs (xt, projection weights) — halves the startup DMA; DMAs
    emitted in first-use order so the first sc-block computes while the rest
    streams in.
  * phase B is sc-major: per 512-token block, q/k/v raw projections, causal
    conv (diagonal-stationary matmuls) + SiLU, squared planes, and the
    token-major scalar matmuls (beta via N=1 matmuls against xt chunks, l2
    sums via N=1 matmuls against squared planes).  sigmoid is computed as
    tanh (same activation table as silu); all sqrts batch at phase-B end so
    the activation table switches exactly once.
  * phase D all-bf16: explicit solve operator U = T^T = I+Z+...+Z^15 built
    from the X/Z power ladder (R1 = (I+Z)(I+Z2), Q2 = (I+X4)(I+X8),
    U = Q2^T R1) with identity adds folded into PSUM accumulation.  The
    sequential chunk chain is one 256-wide matmul w = U^T y.  pass1 is
    emitted rung-interleaved in chunk pairs so the in-order PE never waits
    on the mm->copy->mm ladder.
  * PSUM hand-packed into exactly 8 banks.
"""

import os
import sys
from contextlib import ExitStack

import numpy as np

for _p in ("/opt/trn_rl_repo", "/root/.axon_site/_ro/trn_rl_repo"):
    if os.path.isdir(_p) and _p not in sys.path:
        sys.path.insert(0, _p)

import concourse.bass as bass  # noqa: E402
import concourse.tile as tile  # noqa: E402
from concourse import bacc, mybir  # noqa: E402
from concourse.bass_utils import run_bass_kernel_spmd  # noqa: E402

F32 = mybir.dt.float32
F32R = mybir.dt.float32r
BF16 = mybir.dt.bfloat16
AF = mybir.ActivationFunctionType
OP = mybir.AluOpType

HID = 1024
D = 256
C = 128
KT = HID // 128  # 8 k-tiles over the hidden contraction dim
NH = 4
B = 2
S_FULL = 2048


def build_nc(nchunk=S_FULL // C):
    S = nchunk * C
    scs = 512 if S >= 512 else S
    nsc = S // scs
    cpsc = scs // C  # chunks per sc-block
    nc = bacc.Bacc("TRN2", target_bir_lowering=False, debug=False)

    xt_d = nc.dram_tensor("xt", [HID, S], BF16, kind="ExternalInput")
    wq_d = nc.dram_tensor("wq", [HID, D], BF16, kind="ExternalInput")
    wk_d = nc.dram_tensor("wk", [HID, D], BF16, kind="ExternalInput")
    wv_d = nc.dram_tensor("wv", [HID, D], BF16, kind="ExternalInput")
    wb_d = nc.dram_tensor("wb", [HID, 1], BF16, kind="ExternalInput")
    wo_d = nc.dram_tensor("wo", [D, HID], BF16, kind="ExternalInput")
    cdq_d = nc.dram_tensor("cdq", [128, 8 * 128], F32R, kind="ExternalInput")
    cdk_d = nc.dram_tensor("cdk", [128, 8 * 128], F32R, kind="ExternalInput")
    cdv_d = nc.dram_tensor("cdv", [128, 8 * 128], F32R, kind="ExternalInput")
    identb_d = nc.dram_tensor("identb", [128, 128], BF16, kind="ExternalInput")
    onescol_d = nc.dram_tensor("onescol", [128, 1], BF16, kind="ExternalInput")
    mlow_d = nc.dram_tensor("mlow", [128, 128], F32, kind="ExternalInput")
    mup_d = nc.dram_tensor("mup", [128, 128], F32, kind="ExternalInput")
    out_d = nc.dram_tensor("out", [S, HID], F32, kind="ExternalOutput")

    with tile.TileContext(nc) as tc, ExitStack() as ctx:
        # ---------------- persistent pools ----------------
        pmask = ctx.enter_context(tc.tile_pool(name="pmask", bufs=1))
        pplane = ctx.enter_context(tc.tile_pool(name="pplane", bufs=1))
        pwo = ctx.enter_context(tc.tile_pool(name="pwo", bufs=1))
        ptok = ctx.enter_context(tc.tile_pool(name="ptok", bufs=1))

        identb = pmask.tile([128, 128], BF16)
        onescol = pmask.tile([128, 1], BF16)
        mlow = pmask.tile([128, 128], F32)
        mup = pmask.tile([128, 128], F32)
        eps6 = pmask.tile([128, 1], F32)
        nc.vector.memset(eps6, 1e-6)
        eps5 = pmask.tile([128, 1], F32)
        nc.vector.memset(eps5, 1e-5)

        wo_sb = pwo.tile([128, 2, HID], BF16)

        # q/k/v planes, 2 d-tiles each (post conv+silu)
        planes = {}
        for t in ("q", "k", "v"):
            for dt_ in range(2):
                planes[(t, dt_)] = pplane.tile(
                    [128, S], BF16, tag=f"plane_{t}{dt_}", name=f"plane_{t}{dt_}"
                )

        # per-chunk token scalars: col0 bk=beta*ak, col1 nbk2=-bk*ak,
        # col2 aq, col3 -ak
        toks = [ptok.tile([128, 4], F32, tag=f"tok{i}", name=f"tok{i}")
                for i in range(nchunk)]

        # ---------------- phase B: projections + conv + silu ----------------
        with ExitStack() as bctx:
            pxt = bctx.enter_context(tc.tile_pool(name="pxt", bufs=1))
            pw = bctx.enter_context(tc.tile_pool(name="pw", bufs=1))
            pdiag = bctx.enter_context(tc.tile_pool(name="pdiag", bufs=1))
            praw = bctx.enter_context(tc.tile_pool(name="praw", bufs=1))
            psq = bctx.enter_context(tc.tile_pool(name="psq", bufs=1))
            pbs = bctx.enter_context(tc.tile_pool(name="pbs", bufs=4))
            ppt_b = bctx.enter_context(tc.tile_pool(name="pptb", bufs=5, space="PSUM"))
            ppB = bctx.enter_context(tc.tile_pool(name="ppB", bufs=1, space="PSUM"))

            # beta + l2-sum scalars for all chunks, packed into one psum bank:
            # per chunk i, cols 4i+0 = beta, 4i+1 = sum q^2, 4i+2 = sum k^2
            psBS = ppB.tile([128, 4 * nchunk], F32, tag="psBS", name="psBS")

            xt_sb = pxt.tile([128, KT, S], BF16)
            xt_src = xt_d.ap().rearrange("(k p) s -> p k s", p=128)
            wd_srcs = {"q": wq_d, "k": wk_d, "v": wv_d}
            w_sbs = {}
            for t in ("q", "k", "v"):
                w_sbs[t] = pw.tile([128, KT, D], BF16, tag="w", name=f"w_{t}", bufs=3)
            wb_sb = pw.tile([128, KT, 1], BF16, tag="wb")
            diags = {}
            for t in ("q", "k", "v"):
                diags[t] = pdiag.tile([128, 8 * 128], F32R, tag=f"diag_{t}",
                                      name=f"diag_{t}")
            raw_tiles = {}
            for t in ("q", "k", "v"):
                for dt_ in range(2):
                    raw_tiles[(t, dt_)] = praw.tile(
                        [128, S + 8], F32R, tag=f"raw_{t}{dt_}", name=f"raw_{t}{dt_}"
                    )
            sq_tiles = {}
            for t in ("q", "k"):
                for dt_ in range(2):
                    sq_tiles[(t, dt_)] = psq.tile(
                        [128, S], BF16, tag=f"sq_{t}{dt_}", name=f"sq_{t}{dt_}"
                    )

            # DMAs in first-use order (SP queue drains in emission order)
            def dma_xt_sc(sc):
                sl = slice(sc * scs, (sc + 1) * scs)
                for kk in range(KT):
                    nc.sync.dma_start(out=xt_sb[:, kk, sl], in_=xt_src[:, kk, sl])

            nc.sync.dma_start(
                out=w_sbs["q"], in_=wd_srcs["q"].ap().rearrange("(k p) d -> p k d", p=128)
            )
            nc.sync.dma_start(out=wb_sb,
                              in_=wb_d.ap().rearrange("(k p) o -> p k o", p=128))
            dma_xt_sc(0)
            nc.sync.dma_start(
                out=w_sbs["k"], in_=wd_srcs["k"].ap().rearrange("(k p) d -> p k d", p=128)
            )
            nc.sync.dma_start(out=diags["q"], in_=cdq_d.ap())
            nc.sync.dma_start(
                out=w_sbs["v"], in_=wd_srcs["v"].ap().rearrange("(k p) d -> p k d", p=128)
            )
            nc.sync.dma_start(out=diags["k"], in_=cdk_d.ap())
            nc.sync.dma_start(out=diags["v"], in_=cdv_d.ap())
            nc.sync.dma_start(out=identb, in_=identb_d.ap())
            nc.sync.dma_start(out=onescol, in_=onescol_d.ap())
            nc.sync.dma_start(out=mlow, in_=mlow_d.ap())
            nc.sync.dma_start(out=mup, in_=mup_d.ap())
            if nsc > 1:
                dma_xt_sc(1)
            nc.sync.dma_start(out=wo_sb,
                              in_=wo_d.ap().rearrange("(t p) h -> p t h", p=128))
            for sc in range(2, nsc):
                dma_xt_sc(sc)

            for t in ("q", "k", "v"):
                for dt_ in range(2):
                    nc.gpsimd.memset(raw_tiles[(t, dt_)][:, 0:8].bitcast(F32), 0.0)

            th_l = [None] * nchunk
            copy_flip = 0
            for sc in range(nsc):
                base = sc * scs
                # raw projections for this block
                for t in ("q", "k", "v"):
                    w_sb = w_sbs[t]
                    for dt_ in range(2):
                        raw = raw_tiles[(t, dt_)]
                        ps = ppt_b.tile([128, scs], F32, tag="ps", name="psraw")
                        for kk in range(KT):
                            nc.tensor.matmul(
                                ps,
                                w_sb[:, kk, dt_ * 128 : (dt_ + 1) * 128],
                                xt_sb[:, kk, base : base + scs],
                                start=(kk == 0), stop=(kk == KT - 1),
                            )
                        dst = raw[:, 8 + base : 8 + base + scs]
                        if copy_flip % 2 == 0:
                            nc.scalar.activation(out=dst, in_=ps, func=AF.Copy)
                        else:
                            nc.vector.tensor_copy(dst, ps)
                        copy_flip += 1
                # beta matmuls for the chunks of this block (xt only)
                for i in range(sc * cpsc, (sc + 1) * cpsc):
                    ch = slice(i * C, (i + 1) * C)
                    for kk in range(KT):
                        nc.tensor.matmul(
                            psBS[:, 4 * i : 4 * i + 1], xt_sb[:, kk, ch],
                            wb_sb[:, kk, :],
                            start=(kk == 0), stop=(kk == KT - 1),
                        )
                # conv + silu + squared planes
                for t in ("q", "k", "v"):
                    diag = diags[t]
                    for dt_ in range(2):
                        raw = raw_tiles[(t, dt_)]
                        psc = ppt_b.tile([128, scs], F32, tag="ps", name="psconv")
                        for j in (3, 2, 1, 0):
                            sh = 3 - j
                            dslc = diag[:, (j * 2 + dt_) * 128 : (j * 2 + dt_ + 1) * 128]
                            nc.tensor.matmul(
                                psc, dslc,
                                raw[:, 8 + base - sh : 8 + base + scs - sh],
                                start=(j == 3), stop=(j == 0),
                            )
                        plane = planes[(t, dt_)]
                        nc.scalar.activation(
                            out=plane[:, base : base + scs], in_=psc, func=AF.Silu
                        )
                        if t in ("q", "k"):
                            sqv = sq_tiles[(t, dt_)]
                            nc.gpsimd.tensor_mul(
                                sqv[:, base : base + scs],
                                plane[:, base : base + scs],
                                plane[:, base : base + scs],
                            )
                # l2-sum matmuls + tanh(beta) for the chunks of this block
                for i in range(sc * cpsc, (sc + 1) * cpsc):
                    ch = slice(i * C, (i + 1) * C)
                    for col, t in ((1, "q"), (2, "k")):
                        nc.tensor.matmul(
                            psBS[:, 4 * i + col : 4 * i + col + 1],
                            sq_tiles[(t, 0)][:, ch], onescol,
                            start=True, stop=False,
                        )
                        nc.tensor.matmul(
                            psBS[:, 4 * i + col : 4 * i + col + 1],
                            sq_tiles[(t, 1)][:, ch], onescol,
                            start=False, stop=True,
                        )
                    # beta = (1+tanh(x/2))/2 — tanh shares the silu act table
                    th = pbs.tile([128, 1], F32, tag="th", name=f"th_{i}", bufs=nchunk)
                    nc.scalar.activation(
                        out=th, in_=psBS[:, 4 * i : 4 * i + 1], func=AF.Tanh, scale=0.5
                    )
                    th_l[i] = th

            # batched sqrt/recip token scalars (one act-table switch to sqrt)
            for i in range(nchunk):
                sk_s = pbs.tile([128, 1], F32, tag="sk_s", name="sk_s", bufs=4)
                nc.scalar.activation(
                    out=sk_s, in_=psBS[:, 4 * i + 2 : 4 * i + 3],
                    func=AF.Sqrt, bias=eps6,
                )
                ak_t = pbs.tile([128, 1], F32, tag="ak", name="ak", bufs=4)
                nc.vector.reciprocal(out=ak_t, in_=sk_s)
                # bk = beta*ak = (1+th)/2 * ak
                nc.vector.scalar_tensor_tensor(
                    out=toks[i][:, 0:1], in0=th_l[i], scalar=ak_t,
                    in1=ak_t, op0=OP.mult, op1=OP.add,
                )
                nc.vector.tensor_scalar(
                    out=toks[i][:, 0:1], in0=toks[i][:, 0:1], scalar1=0.5,
                    scalar2=None, op0=OP.mult,
                )
                nc.vector.tensor_scalar(
                    out=toks[i][:, 3:4], in0=ak_t, scalar1=-1.0,
                    scalar2=None, op0=OP.mult,
                )
                nc.vector.tensor_mul(toks[i][:, 1:2], toks[i][:, 0:1],
                                     toks[i][:, 3:4])
                sq_s = pbs.tile([128, 1], F32, tag="sq_s", name="sq_s", bufs=4)
                nc.scalar.activation(
                    out=sq_s, in_=psBS[:, 4 * i + 1 : 4 * i + 2],
                    func=AF.Sqrt, bias=eps6,
                )
                nc.vector.reciprocal(out=toks[i][:, 2:3], in_=sq_s)

        # ---------------- phase D: chunked delta rule ----------------
        # PSUM is 8 banks of [128, 512]-f32; pack manually:
        #   bank psS : persistent state, two 256-wide accum groups
        #   bank psc : psKS [:,0:256] + psW [:,256:512]
        #   bank psoD: pso ping-pong by chunk parity
        #   banks op0/op1: o_proj halves
        #   banks pf0/pf1: pass1 f32 scratch, 8 rotating [128,128] slots
        #   bank pbf : bf16 transpose outs — Z(parity) | V | K | OT regions
        dctx = ExitStack()
        pS = dctx.enter_context(tc.tile_pool(name="pS", bufs=3))
        pcs = dctx.enter_context(tc.tile_pool(name="pcs", bufs=4))
        pcm = dctx.enter_context(tc.tile_pool(name="pcm", bufs=2))
        pout = dctx.enter_context(tc.tile_pool(name="pout", bufs=2))
        pds = dctx.enter_context(tc.tile_pool(name="pds", bufs=4))
        ppd = dctx.enter_context(tc.tile_pool(name="ppd", bufs=1, space="PSUM"))

        psS = ppd.tile([128, 512], F32, tag="psS", name="psS")
        psS0 = psS[:, 0:256]
        psS1 = psS[:, 256:512]
        # psum start_tensor_calc zeroes the whole 2KB bank, which would wipe
        # the sibling state half mid-accumulation; instead zero once and
        # accumulate with start=False for all chunks.
        nc.vector.memset(psS, 0.0)
        psc = ppd.tile([128, 512], F32, tag="psc", name="psc")
        psoD = ppd.tile([128, 512], F32, tag="psoD", name="psoD")
        op_t = [ppd.tile([128, 512], F32, tag=f"op{j}", name=f"op{j}") for j in range(2)]
        pf = [ppd.tile([128, 512], F32, tag=f"pf{j}", name=f"pf{j}") for j in range(2)]
        slots32 = [pf[j][:, s * 128 : (s + 1) * 128] for j in range(2) for s in range(4)]
        _slot_ctr = [0]

        def slot32():
            s = slots32[_slot_ctr[0] % 8]
            _slot_ctr[0] += 1
            return s

        pbf = ppd.tile([128, 1024], BF16, tag="pbf", name="pbf")
        psZ_par = [pbf[:, 0:128], pbf[:, 896:1024]]
        psV_r = pbf[:, 128:384]
        psK_r = pbf[:, 384:640]
        psOT_r = pbf[:, 640:896]

        # token-major K and V for all chunks via bulk DMA transposes
        # (DMA engines are otherwise idle in phase D)
        ptm = dctx.enter_context(tc.tile_pool(name="ptm", bufs=1))
        ktok_all = ptm.tile([128, nchunk, 256], BF16, tag="ktok_all")
        vtok_all = ptm.tile([128, nchunk, 256], BF16, tag="vtok_all")
        nc.sync.dma_start_transpose(out=ktok_all[:, :, 0:128], in_=planes[("k", 0)])
        nc.sync.dma_start_transpose(out=ktok_all[:, :, 128:256], in_=planes[("k", 1)])
        nc.sync.dma_start_transpose(out=vtok_all[:, :, 0:128], in_=planes[("v", 0)])
        nc.sync.dma_start_transpose(out=vtok_all[:, :, 128:256], in_=planes[("v", 1)])

        U_l = [None] * nchunk
        Hm_l = [None] * nchunk

        def pass1_steps(i):
            """List of emit-closures, one per ladder rung, for chunk i."""
            ch = slice(i * C, (i + 1) * C)
            k0 = planes[("k", 0)][:, ch]
            k1 = planes[("k", 1)][:, ch]
            q0 = planes[("q", 0)][:, ch]
            q1 = planes[("q", 1)][:, ch]
            tok = toks[i]
            nbk2_t = tok[:, 1:2]
            t_ = {}

            def sb(name, psrc, eng, keep=False):
                dst = pcs.tile([128, 128], BF16, tag=name, name=f"{name}_{i}",
                               bufs=(nchunk if keep else 8))
                if eng == "a":
                    nc.scalar.activation(out=dst, in_=psrc, func=AF.Copy)
                elif eng == "v":
                    nc.vector.tensor_copy(dst, psrc)
                else:
                    nc.gpsimd.tensor_copy(dst, psrc)
                t_[name] = dst
                return dst

            def s_A():
                psA = slot32()
                nc.tensor.matmul(psA, k0, k0, start=True, stop=False)
                nc.tensor.matmul(psA, k1, k1, start=False, stop=True)
                X = pcs.tile([128, 128], BF16, tag="X", name=f"X_{i}", bufs=8)
                nc.vector.scalar_tensor_tensor(
                    out=X, in0=psA, scalar=nbk2_t, in1=mlow, op0=OP.mult, op1=OP.mult
                )
                t_["X"] = X

            def s_Z():
                psZ = psZ_par[i % 2]
                nc.tensor.transpose(psZ, t_["X"], identb)
                sb("Z", psZ, "a")

            def mk_mm(lhs, rhs, name, eng, keep=False):
                def go():
                    psp = slot32()
                    nc.tensor.matmul(psp, t_[lhs], t_[rhs], start=True, stop=True)
                    sb(name, psp, eng, keep=keep)
                return go

            def s_R1():
                psR1 = slot32()
                nc.tensor.matmul(psR1, identb, identb, start=True, stop=False)
                nc.tensor.matmul(psR1, identb, t_["Z"], start=False, stop=False)
                nc.tensor.matmul(psR1, t_["X2"], identb, start=False, stop=False)
                nc.tensor.matmul(psR1, t_["X2"], t_["Z"], start=False, stop=True)
                sb("R1", psR1, "a")

            def s_Q2():
                psQ2 = slot32()
                nc.tensor.matmul(psQ2, identb, identb, start=True, stop=False)
                nc.tensor.matmul(psQ2, identb, t_["X4"], start=False, stop=False)
                nc.tensor.matmul(psQ2, t_["Z4"], t_["X4"], start=False, stop=False)
                nc.tensor.matmul(psQ2, t_["Z8"], t_["X4"], start=False, stop=True)
                sb("Q2", psQ2, "v")

            def s_U():
                # U scaled by bk along its partitions (= contraction tokens):
                # w = U^T (bk*y') = (bk-row-scaled U)^T y'
                psU = slot32()
                nc.tensor.matmul(psU, t_["Q2"], t_["R1"], start=True, stop=True)
                U = pcs.tile([128, 128], BF16, tag="U", name=f"U_{i}", bufs=nchunk)
                nc.vector.tensor_scalar(
                    out=U, in0=psU, scalar1=tok[:, 0:1], scalar2=None, op0=OP.mult
                )
                U_l[i] = U

            def s_H():
                psH = slot32()
                nc.tensor.matmul(psH, k0, q0, start=True, stop=False)
                nc.tensor.matmul(psH, k1, q1, start=False, stop=True)
                Hm = pcs.tile([128, 128], BF16, tag="Hm", name=f"Hm_{i}", bufs=nchunk)
                nc.vector.tensor_mul(Hm, psH, mup)
                Hm_l[i] = Hm

            return [
                s_A, s_Z,
                mk_mm("Z", "X", "X2", "a"),
                mk_mm("X", "Z", "Z2", "v"),
                mk_mm("Z2", "X2", "X4", "v"),
                mk_mm("X2", "Z2", "Z4", "a"),
                mk_mm("X4", "Z4", "Z8", "v"),
                s_R1, s_Q2, s_U, s_H,
            ]

        chain_state = {}

        def emit_chain(i):
            ch = slice(i * C, (i + 1) * C)
            k0 = planes[("k", 0)][:, ch]
            k1 = planes[("k", 1)][:, ch]
            q0 = planes[("q", 0)][:, ch]
            q1 = planes[("q", 1)][:, ch]
            tok = toks[i]
            nbk2_t = tok[:, 1:2]

            if i > 0:
                S_sb = pS.tile([128, 512], BF16, tag="S", name="S_sb")
                nc.vector.tensor_copy(S_sb[:, 0:256], psS0)
                nc.scalar.activation(out=S_sb[:, 256:512], in_=psS1, func=AF.Copy)
                psKS = psc[:, 0:256]
                nc.tensor.matmul(psKS, k0, S_sb[:, 0:256], start=True, stop=False)
                nc.tensor.matmul(psKS, k1, S_sb[:, 256:512], start=False, stop=True)
                y = pcm.tile([128, 256], BF16, tag="y", name="y", bufs=3)
                nc.vector.scalar_tensor_tensor(
                    out=y, in0=psKS, scalar=tok[:, 3:4], in1=vtok_all[:, i, :],
                    op0=OP.mult, op1=OP.add,
                )
            else:
                S_sb = None
                y = vtok_all[:, i, :]

            # w = U^T y  (U = T^T, so U^T y = T y)
            psW = psc[:, 256:512]
            nc.tensor.matmul(psW, U_l[i], y, start=True, stop=True)
            w = pcm.tile([128, 256], BF16, tag="w", name="w", bufs=3)
            nc.scalar.activation(out=w, in_=psW, func=AF.Copy)

            # state update S += K^T w (bank pre-zeroed; no start flags)
            nc.tensor.matmul(
                psS0, ktok_all[:, i, 0:128], w,
                start=False, stop=(i == nchunk - 1), skip_group_check=True,
            )
            nc.tensor.matmul(
                psS1, ktok_all[:, i, 128:256], w,
                start=False, stop=(i == nchunk - 1), skip_group_check=True,
            )

            # o = Q S + Hm^T w
            pso = psoD[:, (i % 2) * 256 : (i % 2) * 256 + 256]
            if i > 0:
                nc.tensor.matmul(pso, q0, S_sb[:, 0:256], start=True, stop=False)
                nc.tensor.matmul(pso, q1, S_sb[:, 256:512], start=False, stop=False)
                nc.tensor.matmul(pso, Hm_l[i], w, start=False, stop=True)
            else:
                nc.tensor.matmul(pso, Hm_l[i], w, start=True, stop=True)
            chain_state[i] = pso

        tr_state = {}

        def emit_tr1(i):
            tok = toks[i]
            aq_t = tok[:, 2:3]
            pso = chain_state.pop(i)

            # rms-norm on alpha_q-scaled o, scale folded into the `on` copy
            sums = pds.tile([128, 1], F32, tag="sums", name="sums")
            scratch = pcm.tile([128, 256], F32, tag="scr", name="scratch", bufs=4)
            nc.scalar.activation(out=scratch, in_=pso, func=AF.Square, accum_out=sums)
            aq2 = pds.tile([128, 1], F32, tag="aq2", name="aq2")
            nc.vector.tensor_scalar(
                out=aq2, in0=aq_t, scalar1=aq_t, scalar2=1.0 / D,
                op0=OP.mult, op1=OP.mult,
            )
            rstd_t = pds.tile([128, 1], F32, tag="rstd_t", name="rstd_t")
            nc.scalar.activation(
                out=rstd_t, in_=sums, func=AF.Sqrt, scale=aq2[:, 0:1], bias=eps5
            )
            rstd = pds.tile([128, 1], F32, tag="rstd", name="rstd")
            nc.vector.reciprocal(out=rstd, in_=rstd_t)
            saq = pds.tile([128, 1], F32, tag="saq", name="saq")
            nc.vector.tensor_mul(saq, rstd, aq_t)
            on = pcm.tile([128, 256], BF16, tag="on", name="on", bufs=4)
            nc.scalar.activation(
                out=on, in_=pso, func=AF.Copy, scale=saq[:, 0:1]
            )
            tr_state[i] = on

        def emit_tr2(i):
            on = tr_state.pop(i)
            psOT = psOT_r
            nc.tensor.transpose(psOT[:, 0:128], on[:, 0:128], identb)
            nc.tensor.transpose(psOT[:, 128:256], on[:, 128:256], identb)
            ot = pcm.tile([128, 256], BF16, tag="ot", name="ot", bufs=4)
            nc.scalar.activation(out=ot, in_=psOT, func=AF.Copy)
            tr_state[("ot", i)] = ot

        def emit_tr3(i):
            ch = slice(i * C, (i + 1) * C)
            ot = tr_state.pop(("ot", i))
            outbuf = pout.tile([128, HID], F32, tag="outbuf", name="outbuf")
            for hc in range(2):
                psop = op_t[hc]
                nc.tensor.matmul(
                    psop, ot[:, 0:128], wo_sb[:, 0, hc * 512 : (hc + 1) * 512],
                    start=True, stop=False,
                )
                nc.tensor.matmul(
                    psop, ot[:, 128:256], wo_sb[:, 1, hc * 512 : (hc + 1) * 512],
                    start=False, stop=True,
                )
                if hc == 0:
                    nc.scalar.activation(out=outbuf[:, 0:512], in_=psop, func=AF.Copy)
                else:
                    nc.vector.tensor_copy(outbuf[:, 512:1024], psop)
            nc.sync.dma_start(out=out_d.ap()[ch, :], in_=outbuf)

        # diagonal (skewed) schedule: chunk i's stage r emits at clock i + r,
        # so consecutive PE instructions belong to different chunks and the
        # in-order PE queue never waits on a just-issued copy.
        stages = {}
        for i in range(nchunk):
            steps = pass1_steps(i)
            steps.append(lambda i=i: emit_chain(i))
            steps.append(lambda i=i: emit_tr1(i))
            steps.append(lambda i=i: emit_tr2(i))
            steps.append(lambda i=i: emit_tr3(i))
            stages[i] = steps
        M = len(stages[0])
        for c in range(nchunk + M - 1):
            for i in range(max(0, c - M + 1), min(c, nchunk - 1) + 1):
                stages[i][c - i]()
        dctx.close()

    nc.compile()
    return nc


def make_host_inputs(inputs, nchunk=S_FULL // C):
    """Shard + preprocess full inputs into per-core in_maps."""
    import ml_dtypes

    bf16 = ml_dtypes.bfloat16
    S = nchunk * C
    hs = np.ascontiguousarray(np.asarray(inputs["hidden_states"])[:, :S, :]).astype(
        np.float32
    )
    Wq, Wk, Wv = (np.asarray(inputs[k], np.float32) for k in ("Wq", "Wk", "Wv"))
    Wb = np.asarray(inputs["Wb"], np.float32)
    Wo = np.asarray(inputs["Wo"], np.float32)
    nw = np.asarray(inputs["norm_w"], np.float32)
    convs = {
        k: np.asarray(inputs[k], np.float32) for k in ("conv_q", "conv_k", "conv_v")
    }

    identb = np.eye(128, dtype=bf16)
    onescol = np.ones((128, 1), bf16)
    mlow = np.tril(np.ones((128, 128), np.float32), -1)
    mup = np.triu(np.ones((128, 128), np.float32), 0)

    def diag_pack(cw):
        # cw: [256, 4] tap weights for this head -> [128, 8*128]
        out = np.zeros((128, 8 * 128), np.float32)
        for j in range(4):
            for dt_ in range(2):
                blk = np.diag(cw[dt_ * 128 : (dt_ + 1) * 128, j])
                out[:, (j * 2 + dt_) * 128 : (j * 2 + dt_ + 1) * 128] = blk
        return out

    in_maps = []
    for core in range(8):
        b, h = core // 4, core % 4
        hsel = slice(h * D, (h + 1) * D)
        in_maps.append(
            {
                "xt": np.ascontiguousarray(hs[b].T).astype(bf16),
                "wq": np.ascontiguousarray(Wq[:, hsel]).astype(bf16),
                "wk": np.ascontiguousarray(Wk[:, hsel]).astype(bf16),
                "wv": np.ascontiguousarray(Wv[:, hsel]).astype(bf16),
                "wb": np.ascontiguousarray(Wb[:, h : h + 1]).astype(bf16),
                "wo": np.ascontiguousarray((nw[:, None] * Wo[hsel, :]).astype(bf16)),
                "cdq": diag_pack(convs["conv_q"][hsel]),
                "cdk": diag_pack(convs["conv_k"][hsel]),
                "cdv": diag_pack(convs["conv_v"][hsel]),
                "identb": identb,
                "onescol": onescol,
                "mlow": mlow,
                "mup": mup,
            }
        )
    return in_maps


_NC_CACHE = {}


def _get_nc(nchunk):
    if nchunk not in _NC_CACHE:
        _NC_CACHE[nchunk] = build_nc(nchunk)
    return _NC_CACHE[nchunk]


def kernel(**inputs) -> np.ndarray:
    nchunk = S_FULL // C
    nc = _get_nc(nchunk)
    in_maps = make_host_inputs(inputs, nchunk)
    res = run_bass_kernel_spmd(nc, in_maps, core_ids=list(range(8)))
    S = nchunk * C
    out = np.zeros((B, S, HID), np.float32)
    for core in range(8):
        out[core // 4] += res.results[core]["out"]
    return out


# revision 31
# speedup vs baseline: 1.0078x; 1.0078x over previous
"""DeltaNet Trainium2 kernel — 8-core SPMD, one (batch, head) pair per core.

Full inputs -> shard on host -> Bass/Tile kernel per core -> host unshard.

v3 design:
  * bf16 inputs (xt, projection weights) — halves the startup DMA; DMAs
    emitted in first-use order so the first sc-block computes while the rest
    streams in.
  * phase B is sc-major: per 512-token block, q/k/v raw projections, causal
    conv (diagonal-stationary matmuls) + SiLU, squared planes, and the
    token-major scalar matmuls (beta via N=1 matmuls against xt chunks, l2
    sums via N=1 matmuls against squared planes).  sigmoid is computed as
    tanh (same activation table as silu); all sqrts batch at phase-B end so
    the activation table switches exactly once.
  * phase D all-bf16: explicit solve operator U = T^T = I+Z+...+Z^15 built
    from the X/Z power ladder (R1 = (I+Z)(I+Z2), Q2 = (I+X4)(I+X8),
    U = Q2^T R1) with identity adds folded into PSUM accumulation.  The
    sequential chunk chain is one 256-wide matmul w = U^T y.  pass1 is
    emitted rung-interleaved in chunk pairs so the in-order PE never waits
    on the mm->copy->mm ladder.
  * PSUM hand-packed into exactly 8 banks.
"""

import os
import sys
from contextlib import ExitStack

import numpy as np

for _p in ("/opt/trn_rl_repo", "/root/.axon_site/_ro/trn_rl_repo"):
    if os.path.isdir(_p) and _p not in sys.path:
        sys.path.insert(0, _p)

import concourse.bass as bass  # noqa: E402
import concourse.tile as tile  # noqa: E402
from concourse import bacc, mybir  # noqa: E402
from concourse.bass_utils import run_bass_kernel_spmd  # noqa: E402

F32 = mybir.dt.float32
F32R = mybir.dt.float32r
BF16 = mybir.dt.bfloat16
AF = mybir.ActivationFunctionType
OP = mybir.AluOpType

HID = 1024
D = 256
C = 128
KT = HID // 128  # 8 k-tiles over the hidden contraction dim
NH = 4
B = 2
S_FULL = 2048


def build_nc(nchunk=S_FULL // C):
    S = nchunk * C
    scs = 512 if S >= 512 else S
    nsc = S // scs
    cpsc = scs // C  # chunks per sc-block
    nc = bacc.Bacc("TRN2", target_bir_lowering=False, debug=False)

    xt_d = nc.dram_tensor("xt", [HID, S], BF16, kind="ExternalInput")
    wq_d = nc.dram_tensor("wq", [HID, D], BF16, kind="ExternalInput")
    wk_d = nc.dram_tensor("wk", [HID, D], BF16, kind="ExternalInput")
    wv_d = nc.dram_tensor("wv", [HID, D], BF16, kind="ExternalInput")
    wb_d = nc.dram_tensor("wb", [HID, 1], BF16, kind="ExternalInput")
    wo_d = nc.dram_tensor("wo", [D, HID], BF16, kind="ExternalInput")
    cdq_d = nc.dram_tensor("cdq", [128, 8 * 128], F32R, kind="ExternalInput")
    cdk_d = nc.dram_tensor("cdk", [128, 8 * 128], F32R, kind="ExternalInput")
    cdv_d = nc.dram_tensor("cdv", [128, 8 * 128], F32R, kind="ExternalInput")
    identb_d = nc.dram_tensor("identb", [128, 128], BF16, kind="ExternalInput")
    onescol_d = nc.dram_tensor("onescol", [128, 1], BF16, kind="ExternalInput")
    mlow_d = nc.dram_tensor("mlow", [128, 128], F32, kind="ExternalInput")
    mup_d = nc.dram_tensor("mup", [128, 128], F32, kind="ExternalInput")
    out_d = nc.dram_tensor("out", [S, HID], F32, kind="ExternalOutput")

    with tile.TileContext(nc) as tc, ExitStack() as ctx:
        # ---------------- persistent pools ----------------
        pmask = ctx.enter_context(tc.tile_pool(name="pmask", bufs=1))
        pplane = ctx.enter_context(tc.tile_pool(name="pplane", bufs=1))
        pwo = ctx.enter_context(tc.tile_pool(name="pwo", bufs=1))
        ptok = ctx.enter_context(tc.tile_pool(name="ptok", bufs=1))

        identb = pmask.tile([128, 128], BF16)
        onescol = pmask.tile([128, 1], BF16)
        mlow = pmask.tile([128, 128], F32)
        mup = pmask.tile([128, 128], F32)
        eps6 = pmask.tile([128, 1], F32)
        nc.vector.memset(eps6, 1e-6)
        eps5 = pmask.tile([128, 1], F32)
        nc.vector.memset(eps5, 1e-5)

        wo_sb = pwo.tile([128, 2, HID], BF16)

        # q/k/v planes, 2 d-tiles each (post conv+silu)
        planes = {}
        for t in ("q", "k", "v"):
            for dt_ in range(2):
                planes[(t, dt_)] = pplane.tile(
                    [128, S], BF16, tag=f"plane_{t}{dt_}", name=f"plane_{t}{dt_}"
                )

        # per-chunk token scalars: col0 bk=beta*ak, col1 nbk2=-bk*ak,
        # col2 aq, col3 -ak
        toks = [ptok.tile([128, 4], F32, tag=f"tok{i}", name=f"tok{i}")
                for i in range(nchunk)]

        # ---------------- phase B: projections + conv + silu ----------------
        with ExitStack() as bctx:
            pxt = bctx.enter_context(tc.tile_pool(name="pxt", bufs=1))
            pw = bctx.enter_context(tc.tile_pool(name="pw", bufs=1))
            pdiag = bctx.enter_context(tc.tile_pool(name="pdiag", bufs=1))
            praw = bctx.enter_context(tc.tile_pool(name="praw", bufs=1))
            psq = bctx.enter_context(tc.tile_pool(name="psq", bufs=1))
            pbs = bctx.enter_context(tc.tile_pool(name="pbs", bufs=4))
            ppt_b = bctx.enter_context(tc.tile_pool(name="pptb", bufs=5, space="PSUM"))
            ppB = bctx.enter_context(tc.tile_pool(name="ppB", bufs=1, space="PSUM"))

            # beta + l2-sum scalars for all chunks, packed into one psum bank:
            # per chunk i, cols 4i+0 = beta, 4i+1 = sum q^2, 4i+2 = sum k^2
            psBS = ppB.tile([128, 4 * nchunk], F32, tag="psBS", name="psBS")

            xt_sb = pxt.tile([128, KT, S], BF16)
            xt_src = xt_d.ap().rearrange("(k p) s -> p k s", p=128)
            wd_srcs = {"q": wq_d, "k": wk_d, "v": wv_d}
            w_sbs = {}
            for t in ("q", "k", "v"):
                w_sbs[t] = pw.tile([128, KT, D], BF16, tag="w", name=f"w_{t}", bufs=3)
            wb_sb = pw.tile([128, KT, 1], BF16, tag="wb")
            diags = {}
            for t in ("q", "k", "v"):
                diags[t] = pdiag.tile([128, 8 * 128], F32R, tag=f"diag_{t}",
                                      name=f"diag_{t}")
            raw_tiles = {}
            for t in ("q", "k", "v"):
                for dt_ in range(2):
                    raw_tiles[(t, dt_)] = praw.tile(
                        [128, S + 8], F32R, tag=f"raw_{t}{dt_}", name=f"raw_{t}{dt_}"
                    )
            sq_tiles = {}
            for t in ("q", "k"):
                for dt_ in range(2):
                    sq_tiles[(t, dt_)] = psq.tile(
                        [128, S], BF16, tag=f"sq_{t}{dt_}", name=f"sq_{t}{dt_}"
                    )

            # DMAs in first-use order (SP queue drains in emission order)
            def dma_xt_sc(sc):
                sl = slice(sc * scs, (sc + 1) * scs)
                for kk in range(KT):
                    nc.sync.dma_start(out=xt_sb[:, kk, sl], in_=xt_src[:, kk, sl])

            nc.sync.dma_start(
                out=w_sbs["q"], in_=wd_srcs["q"].ap().rearrange("(k p) d -> p k d", p=128)
            )
            nc.sync.dma_start(out=wb_sb,
                              in_=wb_d.ap().rearrange("(k p) o -> p k o", p=128))
            dma_xt_sc(0)
            nc.sync.dma_start(
                out=w_sbs["k"], in_=wd_srcs["k"].ap().rearrange("(k p) d -> p k d", p=128)
            )
            nc.sync.dma_start(out=diags["q"], in_=cdq_d.ap())
            nc.sync.dma_start(
                out=w_sbs["v"], in_=wd_srcs["v"].ap().rearrange("(k p) d -> p k d", p=128)
            )
            nc.sync.dma_start(out=diags["k"], in_=cdk_d.ap())
            nc.sync.dma_start(out=diags["v"], in_=cdv_d.ap())
            nc.sync.dma_start(out=identb, in_=identb_d.ap())
            nc.sync.dma_start(out=onescol, in_=onescol_d.ap())
            nc.sync.dma_start(out=mlow, in_=mlow_d.ap())
            nc.sync.dma_start(out=mup, in_=mup_d.ap())
            if nsc > 1:
                dma_xt_sc(1)
            nc.sync.dma_start(out=wo_sb,
                              in_=wo_d.ap().rearrange("(t p) h -> p t h", p=128))
            for sc in range(2, nsc):
                dma_xt_sc(sc)

            for t in ("q", "k", "v"):
                for dt_ in range(2):
                    nc.gpsimd.memset(raw_tiles[(t, dt_)][:, 0:8].bitcast(F32), 0.0)

            th_l = [None] * nchunk
            copy_flip = 0
            for sc in range(nsc):
                base = sc * scs
                # raw projections for this block
                for t in ("q", "k", "v"):
                    w_sb = w_sbs[t]
                    for dt_ in range(2):
                        raw = raw_tiles[(t, dt_)]
                        ps = ppt_b.tile([128, scs], F32, tag="ps", name="psraw")
                        for kk in range(KT):
                            nc.tensor.matmul(
                                ps,
                                w_sb[:, kk, dt_ * 128 : (dt_ + 1) * 128],
                                xt_sb[:, kk, base : base + scs],
                                start=(kk == 0), stop=(kk == KT - 1),
                            )
                        dst = raw[:, 8 + base : 8 + base + scs]
                        if copy_flip % 2 == 0:
                            nc.scalar.activation(out=dst, in_=ps, func=AF.Copy)
                        else:
                            nc.vector.tensor_copy(dst, ps)
                        copy_flip += 1
                # beta matmuls for the chunks of this block (xt only)
                for i in range(sc * cpsc, (sc + 1) * cpsc):
                    ch = slice(i * C, (i + 1) * C)
                    for kk in range(KT):
                        nc.tensor.matmul(
                            psBS[:, 4 * i : 4 * i + 1], xt_sb[:, kk, ch],
                            wb_sb[:, kk, :],
                            start=(kk == 0), stop=(kk == KT - 1),
                        )
                # conv + silu + squared planes
                for t in ("q", "k", "v"):
                    diag = diags[t]
                    for dt_ in range(2):
                        raw = raw_tiles[(t, dt_)]
                        psc = ppt_b.tile([128, scs], F32, tag="ps", name="psconv")
                        for j in (3, 2, 1, 0):
                            sh = 3 - j
                            dslc = diag[:, (j * 2 + dt_) * 128 : (j * 2 + dt_ + 1) * 128]
                            nc.tensor.matmul(
                                psc, dslc,
                                raw[:, 8 + base - sh : 8 + base + scs - sh],
                                start=(j == 3), stop=(j == 0),
                            )
                        plane = planes[(t, dt_)]
                        nc.scalar.activation(
                            out=plane[:, base : base + scs], in_=psc, func=AF.Silu
                        )
                        if t in ("q", "k"):
                            sqv = sq_tiles[(t, dt_)]
                            nc.gpsimd.tensor_mul(
                                sqv[:, base : base + scs],
                                plane[:, base : base + scs],
                                plane[:, base : base + scs],
                            )
                # l2-sum matmuls + tanh(beta) for the chunks of this block
                for i in range(sc * cpsc, (sc + 1) * cpsc):
                    ch = slice(i * C, (i + 1) * C)
                    for col, t in ((1, "q"), (2, "k")):
                        nc.tensor.matmul(
                            psBS[:, 4 * i + col : 4 * i + col + 1],
                            sq_tiles[(t, 0)][:, ch], onescol,
                            start=True, stop=False,
                        )
                        nc.tensor.matmul(
                            psBS[:, 4 * i + col : 4 * i + col + 1],
                            sq_tiles[(t, 1)][:, ch], onescol,
                            start=False, stop=True,
                        )
                    # beta = (1+tanh(x/2))/2 — tanh shares the silu act table
                    th = pbs.tile([128, 1], F32, tag="th", name=f"th_{i}", bufs=nchunk)
                    nc.scalar.activation(
                        out=th, in_=psBS[:, 4 * i : 4 * i + 1], func=AF.Tanh, scale=0.5
                    )
                    th_l[i] = th

            # batched sqrt/recip token scalars (one act-table switch to sqrt)
            for i in range(nchunk):
                sk_s = pbs.tile([128, 1], F32, tag="sk_s", name="sk_s", bufs=4)
                nc.scalar.activation(
                    out=sk_s, in_=psBS[:, 4 * i + 2 : 4 * i + 3],
                    func=AF.Sqrt, bias=eps6,
                )
                ak_t = pbs.tile([128, 1], F32, tag="ak", name="ak", bufs=4)
                nc.vector.reciprocal(out=ak_t, in_=sk_s)
                # bk = beta*ak = (1+th)/2 * ak
                nc.vector.scalar_tensor_tensor(
                    out=toks[i][:, 0:1], in0=th_l[i], scalar=ak_t,
                    in1=ak_t, op0=OP.mult, op1=OP.add,
                )
                nc.vector.tensor_scalar(
                    out=toks[i][:, 0:1], in0=toks[i][:, 0:1], scalar1=0.5,
                    scalar2=None, op0=OP.mult,
                )
                nc.vector.tensor_scalar(
                    out=toks[i][:, 3:4], in0=ak_t, scalar1=-1.0,
                    scalar2=None, op0=OP.mult,
                )
                nc.vector.tensor_mul(toks[i][:, 1:2], toks[i][:, 0:1],
                                     toks[i][:, 3:4])
                sq_s = pbs.tile([128, 1], F32, tag="sq_s", name="sq_s", bufs=4)
                nc.scalar.activation(
                    out=sq_s, in_=psBS[:, 4 * i + 1 : 4 * i + 2],
                    func=AF.Sqrt, bias=eps6,
                )
                nc.vector.reciprocal(out=toks[i][:, 2:3], in_=sq_s)

        # ---------------- phase D: chunked delta rule ----------------
        # PSUM is 8 banks of [128, 512]-f32; pack manually:
        #   bank psS : persistent state, two 256-wide accum groups
        #   bank psc : psKS [:,0:256] + psW [:,256:512]
        #   bank psoD: pso ping-pong by chunk parity
        #   banks op0/op1: o_proj halves
        #   banks pf0/pf1: pass1 f32 scratch, 8 rotating [128,128] slots
        #   bank pbf : bf16 transpose outs — Z(parity) | V | K | OT regions
        dctx = ExitStack()
        pS = dctx.enter_context(tc.tile_pool(name="pS", bufs=3))
        pcs = dctx.enter_context(tc.tile_pool(name="pcs", bufs=4))
        pcm = dctx.enter_context(tc.tile_pool(name="pcm", bufs=2))
        pout = dctx.enter_context(tc.tile_pool(name="pout", bufs=2))
        pds = dctx.enter_context(tc.tile_pool(name="pds", bufs=4))
        ppd = dctx.enter_context(tc.tile_pool(name="ppd", bufs=1, space="PSUM"))

        psS = ppd.tile([128, 512], F32, tag="psS", name="psS")
        psS0 = psS[:, 0:256]
        psS1 = psS[:, 256:512]
        # psum start_tensor_calc zeroes the whole 2KB bank, which would wipe
        # the sibling state half mid-accumulation; instead zero once and
        # accumulate with start=False for all chunks.
        nc.vector.memset(psS, 0.0)
        psc = ppd.tile([128, 512], F32, tag="psc", name="psc")
        psoD = ppd.tile([128, 512], F32, tag="psoD", name="psoD")
        op_t = [ppd.tile([128, 512], F32, tag=f"op{j}", name=f"op{j}") for j in range(2)]
        pf = [ppd.tile([128, 512], F32, tag=f"pf{j}", name=f"pf{j}") for j in range(2)]
        slots32 = [pf[j][:, s * 128 : (s + 1) * 128] for j in range(2) for s in range(4)]
        _slot_ctr = [0]

        def slot32():
            s = slots32[_slot_ctr[0] % 8]
            _slot_ctr[0] += 1
            return s

        pbf = ppd.tile([128, 1024], BF16, tag="pbf", name="pbf")
        psZ_par = [pbf[:, 0:128], pbf[:, 896:1024]]
        psV_r = pbf[:, 128:384]
        psK_r = pbf[:, 384:640]
        psOT_r = pbf[:, 640:896]

        # token-major K and V for all chunks via bulk DMA transposes
        # (DMA engines are otherwise idle in phase D)
        ptm = dctx.enter_context(tc.tile_pool(name="ptm", bufs=1))
        ktok_all = ptm.tile([128, nchunk, 256], BF16, tag="ktok_all")
        vtok_all = ptm.tile([128, nchunk, 256], BF16, tag="vtok_all")
        nc.sync.dma_start_transpose(out=ktok_all[:, :, 0:128], in_=planes[("k", 0)])
        nc.sync.dma_start_transpose(out=ktok_all[:, :, 128:256], in_=planes[("k", 1)])
        nc.sync.dma_start_transpose(out=vtok_all[:, :, 0:128], in_=planes[("v", 0)])
        nc.sync.dma_start_transpose(out=vtok_all[:, :, 128:256], in_=planes[("v", 1)])

        U_l = [None] * nchunk
        Hm_l = [None] * nchunk

        def pass1_steps(i):
            """List of emit-closures, one per ladder rung, for chunk i."""
            ch = slice(i * C, (i + 1) * C)
            k0 = planes[("k", 0)][:, ch]
            k1 = planes[("k", 1)][:, ch]
            q0 = planes[("q", 0)][:, ch]
            q1 = planes[("q", 1)][:, ch]
            tok = toks[i]
            nbk2_t = tok[:, 1:2]
            t_ = {}

            def sb(name, psrc, eng, keep=False):
                dst = pcs.tile([128, 128], BF16, tag=name, name=f"{name}_{i}",
                               bufs=(nchunk if keep else 8))
                if eng == "a":
                    nc.scalar.activation(out=dst, in_=psrc, func=AF.Copy)
                elif eng == "v":
                    nc.vector.tensor_copy(dst, psrc)
                else:
                    nc.gpsimd.tensor_copy(dst, psrc)
                t_[name] = dst
                return dst

            def s_A():
                psA = slot32()
                nc.tensor.matmul(psA, k0, k0, start=True, stop=False)
                nc.tensor.matmul(psA, k1, k1, start=False, stop=True)
                X = pcs.tile([128, 128], BF16, tag="X", name=f"X_{i}", bufs=8)
                nc.vector.scalar_tensor_tensor(
                    out=X, in0=psA, scalar=nbk2_t, in1=mlow, op0=OP.mult, op1=OP.mult
                )
                t_["X"] = X

            def s_Z():
                psZ = psZ_par[i % 2]
                nc.tensor.transpose(psZ, t_["X"], identb)
                sb("Z", psZ, "a")

            def mk_mm(lhs, rhs, name, eng, keep=False):
                def go():
                    psp = slot32()
                    nc.tensor.matmul(psp, t_[lhs], t_[rhs], start=True, stop=True)
                    sb(name, psp, eng, keep=keep)
                return go

            def s_R1():
                psR1 = slot32()
                nc.tensor.matmul(psR1, identb, identb, start=True, stop=False)
                nc.tensor.matmul(psR1, identb, t_["Z"], start=False, stop=False)
                nc.tensor.matmul(psR1, t_["X2"], identb, start=False, stop=False)
                nc.tensor.matmul(psR1, t_["X2"], t_["Z"], start=False, stop=True)
                sb("R1", psR1, "a")

            def s_Q2():
                psQ2 = slot32()
                nc.tensor.matmul(psQ2, identb, identb, start=True, stop=False)
                nc.tensor.matmul(psQ2, identb, t_["X4"], start=False, stop=False)
                nc.tensor.matmul(psQ2, t_["Z4"], t_["X4"], start=False, stop=False)
                nc.tensor.matmul(psQ2, t_["Z8"], t_["X4"], start=False, stop=True)
                sb("Q2", psQ2, "v")

            def s_U():
                # U scaled by bk along its partitions (= contraction tokens):
                # w = U^T (bk*y') = (bk-row-scaled U)^T y'
                psU = slot32()
                nc.tensor.matmul(psU, t_["Q2"], t_["R1"], start=True, stop=True)
                U = pcs.tile([128, 128], BF16, tag="U", name=f"U_{i}", bufs=nchunk)
                nc.vector.tensor_scalar(
                    out=U, in0=psU, scalar1=tok[:, 0:1], scalar2=None, op0=OP.mult
                )
                U_l[i] = U

            def s_H():
                psH = slot32()
                nc.tensor.matmul(psH, k0, q0, start=True, stop=False)
                nc.tensor.matmul(psH, k1, q1, start=False, stop=True)
                Hm = pcs.tile([128, 128], BF16, tag="Hm", name=f"Hm_{i}", bufs=nchunk)
                nc.vector.tensor_mul(Hm, psH, mup)
                Hm_l[i] = Hm

            return [
                s_A, s_Z,
                mk_mm("Z", "X", "X2", "a"),
                mk_mm("X", "Z", "Z2", "v"),
                mk_mm("Z2", "X2", "X4", "v"),
                mk_mm("X2", "Z2", "Z4", "a"),
                mk_mm("X4", "Z4", "Z8", "v"),
                s_R1, s_Q2, s_U, s_H,
            ]

        chain_state = {}

        def emit_chain(i):
            ch = slice(i * C, (i + 1) * C)
            k0 = planes[("k", 0)][:, ch]
            k1 = planes[("k", 1)][:, ch]
            q0 = planes[("q", 0)][:, ch]
            q1 = planes[("q", 1)][:, ch]
            tok = toks[i]
            nbk2_t = tok[:, 1:2]

            if i > 0:
                S_sb = pS.tile([128, 512], BF16, tag="S", name="S_sb")
                nc.vector.tensor_copy(S_sb[:, 0:256], psS0)
                nc.vector.tensor_copy(S_sb[:, 256:512], psS1)
                psKS = psc[:, 0:256]
                nc.tensor.matmul(psKS, k0, S_sb[:, 0:256], start=True, stop=False)
                nc.tensor.matmul(psKS, k1, S_sb[:, 256:512], start=False, stop=True)
                y = pcm.tile([128, 256], BF16, tag="y", name="y", bufs=3)
                nc.vector.scalar_tensor_tensor(
                    out=y, in0=psKS, scalar=tok[:, 3:4], in1=vtok_all[:, i, :],
                    op0=OP.mult, op1=OP.add,
                )
            else:
                S_sb = None
                y = vtok_all[:, i, :]

            # w = U^T y  (U = T^T, so U^T y = T y)
            psW = psc[:, 256:512]
            nc.tensor.matmul(psW, U_l[i], y, start=True, stop=True)
            w = pcm.tile([128, 256], BF16, tag="w", name="w", bufs=3)
            nc.vector.tensor_copy(w, psW)

            # state update S += K^T w (bank pre-zeroed; no start flags)
            nc.tensor.matmul(
                psS0, ktok_all[:, i, 0:128], w,
                start=False, stop=(i == nchunk - 1), skip_group_check=True,
            )
            nc.tensor.matmul(
                psS1, ktok_all[:, i, 128:256], w,
                start=False, stop=(i == nchunk - 1), skip_group_check=True,
            )

            # o = Q S + Hm^T w
            pso = psoD[:, (i % 2) * 256 : (i % 2) * 256 + 256]
            if i > 0:
                nc.tensor.matmul(pso, q0, S_sb[:, 0:256], start=True, stop=False)
                nc.tensor.matmul(pso, q1, S_sb[:, 256:512], start=False, stop=False)
                nc.tensor.matmul(pso, Hm_l[i], w, start=False, stop=True)
            else:
                nc.tensor.matmul(pso, Hm_l[i], w, start=True, stop=True)
            chain_state[i] = pso

        tr_state = {}

        def emit_tr1(i):
            tok = toks[i]
            aq_t = tok[:, 2:3]
            pso = chain_state.pop(i)

            # rms-norm on alpha_q-scaled o, scale folded into the `on` copy
            sums = pds.tile([128, 1], F32, tag="sums", name="sums")
            scratch = pcm.tile([128, 256], F32, tag="scr", name="scratch", bufs=4)
            nc.scalar.activation(out=scratch, in_=pso, func=AF.Square, accum_out=sums)
            aq2 = pds.tile([128, 1], F32, tag="aq2", name="aq2")
            nc.vector.tensor_scalar(
                out=aq2, in0=aq_t, scalar1=aq_t, scalar2=1.0 / D,
                op0=OP.mult, op1=OP.mult,
            )
            rstd_t = pds.tile([128, 1], F32, tag="rstd_t", name="rstd_t")
            nc.scalar.activation(
                out=rstd_t, in_=sums, func=AF.Sqrt, scale=aq2[:, 0:1], bias=eps5
            )
            rstd = pds.tile([128, 1], F32, tag="rstd", name="rstd")
            nc.vector.reciprocal(out=rstd, in_=rstd_t)
            saq = pds.tile([128, 1], F32, tag="saq", name="saq")
            nc.vector.tensor_mul(saq, rstd, aq_t)
            on = pcm.tile([128, 256], BF16, tag="on", name="on", bufs=4)
            nc.scalar.activation(
                out=on, in_=pso, func=AF.Copy, scale=saq[:, 0:1]
            )
            tr_state[i] = on

        def emit_tr2(i):
            on = tr_state.pop(i)
            psOT = psOT_r
            nc.tensor.transpose(psOT[:, 0:128], on[:, 0:128], identb)
            nc.tensor.transpose(psOT[:, 128:256], on[:, 128:256], identb)
            ot = pcm.tile([128, 256], BF16, tag="ot", name="ot", bufs=4)
            nc.scalar.activation(out=ot, in_=psOT, func=AF.Copy)
            tr_state[("ot", i)] = ot

        def emit_tr3(i):
            ch = slice(i * C, (i + 1) * C)
            ot = tr_state.pop(("ot", i))
            outbuf = pout.tile([128, HID], F32, tag="outbuf", name="outbuf")
            for hc in range(2):
                psop = op_t[hc]
                nc.tensor.matmul(
                    psop, ot[:, 0:128], wo_sb[:, 0, hc * 512 : (hc + 1) * 512],
                    start=True, stop=False,
                )
                nc.tensor.matmul(
                    psop, ot[:, 128:256], wo_sb[:, 1, hc * 512 : (hc + 1) * 512],
                    start=False, stop=True,
                )
                if hc == 0:
                    nc.scalar.activation(out=outbuf[:, 0:512], in_=psop, func=AF.Copy)
                else:
                    nc.vector.tensor_copy(outbuf[:, 512:1024], psop)
            nc.sync.dma_start(out=out_d.ap()[ch, :], in_=outbuf)

        # diagonal (skewed) schedule: chunk i's stage r emits at clock i + r,
        # so consecutive PE instructions belong to different chunks and the
        # in-order PE queue never waits on a just-issued copy.
        stages = {}
        for i in range(nchunk):
            steps = pass1_steps(i)
            steps.append(lambda i=i: emit_chain(i))
            steps.append(lambda i=i: emit_tr1(i))
            steps.append(lambda i=i: emit_tr2(i))
            steps.append(lambda i=i: emit_tr3(i))
            stages[i] = steps
        M = len(stages[0])
        for c in range(nchunk + M - 1):
            for i in range(max(0, c - M + 1), min(c, nchunk - 1) + 1):
                stages[i][c - i]()
        dctx.close()

    nc.compile()
    return nc


def make_host_inputs(inputs, nchunk=S_FULL // C):
    """Shard + preprocess full inputs into per-core in_maps."""
    import ml_dtypes

    bf16 = ml_dtypes.bfloat16
    S = nchunk * C
    hs = np.ascontiguousarray(np.asarray(inputs["hidden_states"])[:, :S, :]).astype(
        np.float32
    )
    Wq, Wk, Wv = (np.asarray(inputs[k], np.float32) for k in ("Wq", "Wk", "Wv"))
    Wb = np.asarray(inputs["Wb"], np.float32)
    Wo = np.asarray(inputs["Wo"], np.float32)
    nw = np.asarray(inputs["norm_w"], np.float32)
    convs = {
        k: np.asarray(inputs[k], np.float32) for k in ("conv_q", "conv_k", "conv_v")
    }

    identb = np.eye(128, dtype=bf16)
    onescol = np.ones((128, 1), bf16)
    mlow = np.tril(np.ones((128, 128), np.float32), -1)
    mup = np.triu(np.ones((128, 128), np.float32), 0)

    def diag_pack(cw):
        # cw: [256, 4] tap weights for this head -> [128, 8*128]
        out = np.zeros((128, 8 * 128), np.float32)
        for j in range(4):
            for dt_ in range(2):
                blk = np.diag(cw[dt_ * 128 : (dt_ + 1) * 128, j])
                out[:, (j * 2 + dt_) * 128 : (j * 2 + dt_ + 1) * 128] = blk
        return out

    in_maps = []
    for core in range(8):
        b, h = core // 4, core % 4
        hsel = slice(h * D, (h + 1) * D)
        in_maps.append(
            {
                "xt": np.ascontiguousarray(hs[b].T).astype(bf16),
                "wq": np.ascontiguousarray(Wq[:, hsel]).astype(bf16),
                "wk": np.ascontiguousarray(Wk[:, hsel]).astype(bf16),
                "wv": np.ascontiguousarray(Wv[:, hsel]).astype(bf16),
                "wb": np.ascontiguousarray(Wb[:, h : h + 1]).astype(bf16),
                "wo": np.ascontiguousarray((nw[:, None] * Wo[hsel, :]).astype(bf16)),
                "cdq": diag_pack(convs["conv_q"][hsel]),
                "cdk": diag_pack(convs["conv_k"][hsel]),
                "cdv": diag_pack(convs["conv_v"][hsel]),
                "identb": identb,
                "onescol": onescol,
                "mlow": mlow,
                "mup": mup,
            }
        )
    return in_maps


_NC_CACHE = {}


def _get_nc(nchunk):
    if nchunk not in _NC_CACHE:
        _NC_CACHE[nchunk] = build_nc(nchunk)
    return _NC_CACHE[nchunk]


def kernel(**inputs) -> np.ndarray:
    nchunk = S_FULL // C
    nc = _get_nc(nchunk)
    in_maps = make_host_inputs(inputs, nchunk)
    res = run_bass_kernel_spmd(nc, in_maps, core_ids=list(range(8)))
    S = nchunk * C
    out = np.zeros((B, S, HID), np.float32)
    for core in range(8):
        out[core // 4] += res.results[core]["out"]
    return out


# revision 34
# speedup vs baseline: 1.0195x; 1.0116x over previous
"""DeltaNet Trainium2 kernel — 8-core SPMD, one (batch, head) pair per core.

Full inputs -> shard on host -> Bass/Tile kernel per core -> host unshard.

v3 design:
  * bf16 inputs (xt, projection weights) — halves the startup DMA; DMAs
    emitted in first-use order so the first sc-block computes while the rest
    streams in.
  * phase B is sc-major: per 512-token block, q/k/v raw projections, causal
    conv (diagonal-stationary matmuls) + SiLU, squared planes, and the
    token-major scalar matmuls (beta via N=1 matmuls against xt chunks, l2
    sums via N=1 matmuls against squared planes).  sigmoid is computed as
    tanh (same activation table as silu); all sqrts batch at phase-B end so
    the activation table switches exactly once.
  * phase D all-bf16: explicit solve operator U = T^T = I+Z+...+Z^15 built
    from the X/Z power ladder (R1 = (I+Z)(I+Z2), Q2 = (I+X4)(I+X8),
    U = Q2^T R1) with identity adds folded into PSUM accumulation.  The
    sequential chunk chain is one 256-wide matmul w = U^T y.  pass1 is
    emitted rung-interleaved in chunk pairs so the in-order PE never waits
    on the mm->copy->mm ladder.
  * PSUM hand-packed into exactly 8 banks.
"""

import os
import sys
from contextlib import ExitStack

import numpy as np

for _p in ("/opt/trn_rl_repo", "/root/.axon_site/_ro/trn_rl_repo"):
    if os.path.isdir(_p) and _p not in sys.path:
        sys.path.insert(0, _p)

import concourse.bass as bass  # noqa: E402
import concourse.tile as tile  # noqa: E402
from concourse import bacc, mybir  # noqa: E402
from concourse.bass_utils import run_bass_kernel_spmd  # noqa: E402

F32 = mybir.dt.float32
F32R = mybir.dt.float32r
BF16 = mybir.dt.bfloat16
AF = mybir.ActivationFunctionType
OP = mybir.AluOpType

HID = 1024
D = 256
C = 128
KT = HID // 128  # 8 k-tiles over the hidden contraction dim
NH = 4
B = 2
S_FULL = 2048


def build_nc(nchunk=S_FULL // C):
    S = nchunk * C
    scs = 512 if S >= 512 else S
    nsc = S // scs
    cpsc = scs // C  # chunks per sc-block
    nc = bacc.Bacc("TRN2", target_bir_lowering=False, debug=False)

    xt_d = nc.dram_tensor("xt", [HID, S], BF16, kind="ExternalInput")
    wq_d = nc.dram_tensor("wq", [HID, D], BF16, kind="ExternalInput")
    wk_d = nc.dram_tensor("wk", [HID, D], BF16, kind="ExternalInput")
    wv_d = nc.dram_tensor("wv", [HID, D], BF16, kind="ExternalInput")
    wb_d = nc.dram_tensor("wb", [HID, 1], BF16, kind="ExternalInput")
    wo_d = nc.dram_tensor("wo", [D, HID], BF16, kind="ExternalInput")
    cdq_d = nc.dram_tensor("cdq", [128, 8 * 128], F32R, kind="ExternalInput")
    cdk_d = nc.dram_tensor("cdk", [128, 8 * 128], F32R, kind="ExternalInput")
    cdv_d = nc.dram_tensor("cdv", [128, 8 * 128], F32R, kind="ExternalInput")
    identb_d = nc.dram_tensor("identb", [128, 128], BF16, kind="ExternalInput")
    onescol_d = nc.dram_tensor("onescol", [128, 1], BF16, kind="ExternalInput")
    mlow_d = nc.dram_tensor("mlow", [128, 128], F32, kind="ExternalInput")
    mup_d = nc.dram_tensor("mup", [128, 128], F32, kind="ExternalInput")
    out_d = nc.dram_tensor("out", [S, HID], F32, kind="ExternalOutput")

    with tile.TileContext(nc) as tc, ExitStack() as ctx:
        # ---------------- persistent pools ----------------
        pmask = ctx.enter_context(tc.tile_pool(name="pmask", bufs=1))
        pplane = ctx.enter_context(tc.tile_pool(name="pplane", bufs=1))
        pwo = ctx.enter_context(tc.tile_pool(name="pwo", bufs=1))
        ptok = ctx.enter_context(tc.tile_pool(name="ptok", bufs=1))

        identb = pmask.tile([128, 128], BF16)
        onescol = pmask.tile([128, 1], BF16)
        mlow = pmask.tile([128, 128], F32)
        mup = pmask.tile([128, 128], F32)
        eps6 = pmask.tile([128, 1], F32)
        nc.vector.memset(eps6, 1e-6)
        eps5 = pmask.tile([128, 1], F32)
        nc.vector.memset(eps5, 1e-5)

        wo_sb = pwo.tile([128, 2, HID], BF16)

        # q/k/v planes, 2 d-tiles each (post conv+silu)
        planes = {}
        for t in ("q", "k", "v"):
            for dt_ in range(2):
                planes[(t, dt_)] = pplane.tile(
                    [128, S], BF16, tag=f"plane_{t}{dt_}", name=f"plane_{t}{dt_}"
                )

        # per-chunk token scalars: col0 bk=beta*ak, col1 nbk2=-bk*ak,
        # col2 aq, col3 -ak
        toks = [ptok.tile([128, 4], F32, tag=f"tok{i}", name=f"tok{i}")
                for i in range(nchunk)]

        # ---------------- phase B: projections + conv + silu ----------------
        with ExitStack() as bctx:
            pxt = bctx.enter_context(tc.tile_pool(name="pxt", bufs=1))
            pw = bctx.enter_context(tc.tile_pool(name="pw", bufs=1))
            pdiag = bctx.enter_context(tc.tile_pool(name="pdiag", bufs=1))
            praw = bctx.enter_context(tc.tile_pool(name="praw", bufs=1))
            psq = bctx.enter_context(tc.tile_pool(name="psq", bufs=1))
            pbs = bctx.enter_context(tc.tile_pool(name="pbs", bufs=4))
            ppt_b = bctx.enter_context(tc.tile_pool(name="pptb", bufs=5, space="PSUM"))
            ppB = bctx.enter_context(tc.tile_pool(name="ppB", bufs=1, space="PSUM"))

            # beta + l2-sum scalars for all chunks, packed into one psum bank:
            # per chunk i, cols 4i+0 = beta, 4i+1 = sum q^2, 4i+2 = sum k^2
            psBS = ppB.tile([128, 4 * nchunk], F32, tag="psBS", name="psBS")

            xt_sb = pxt.tile([128, KT, S], BF16)
            xt_src = xt_d.ap().rearrange("(k p) s -> p k s", p=128)
            wd_srcs = {"q": wq_d, "k": wk_d, "v": wv_d}
            w_sbs = {}
            for t in ("q", "k", "v"):
                w_sbs[t] = pw.tile([128, KT, D], BF16, tag="w", name=f"w_{t}", bufs=3)
            wb_sb = pw.tile([128, KT, 1], BF16, tag="wb")
            diags = {}
            for t in ("q", "k", "v"):
                diags[t] = pdiag.tile([128, 8 * 128], F32R, tag=f"diag_{t}",
                                      name=f"diag_{t}")
            raw_tiles = {}
            for t in ("q", "k", "v"):
                for dt_ in range(2):
                    raw_tiles[(t, dt_)] = praw.tile(
                        [128, S + 8], F32R, tag=f"raw_{t}{dt_}", name=f"raw_{t}{dt_}"
                    )
            sq_tiles = {}
            for t in ("q", "k"):
                for dt_ in range(2):
                    sq_tiles[(t, dt_)] = psq.tile(
                        [128, S], BF16, tag=f"sq_{t}{dt_}", name=f"sq_{t}{dt_}"
                    )

            # DMAs in first-use order (SP queue drains in emission order)
            def dma_xt_sc(sc):
                sl = slice(sc * scs, (sc + 1) * scs)
                for kk in range(KT):
                    nc.sync.dma_start(out=xt_sb[:, kk, sl], in_=xt_src[:, kk, sl])

            nc.sync.dma_start(
                out=w_sbs["q"], in_=wd_srcs["q"].ap().rearrange("(k p) d -> p k d", p=128)
            )
            nc.sync.dma_start(out=wb_sb,
                              in_=wb_d.ap().rearrange("(k p) o -> p k o", p=128))
            dma_xt_sc(0)
            nc.sync.dma_start(
                out=w_sbs["k"], in_=wd_srcs["k"].ap().rearrange("(k p) d -> p k d", p=128)
            )
            nc.sync.dma_start(out=diags["q"], in_=cdq_d.ap())
            nc.sync.dma_start(
                out=w_sbs["v"], in_=wd_srcs["v"].ap().rearrange("(k p) d -> p k d", p=128)
            )
            nc.sync.dma_start(out=diags["k"], in_=cdk_d.ap())
            nc.sync.dma_start(out=diags["v"], in_=cdv_d.ap())
            nc.sync.dma_start(out=identb, in_=identb_d.ap())
            nc.sync.dma_start(out=onescol, in_=onescol_d.ap())
            nc.sync.dma_start(out=mlow, in_=mlow_d.ap())
            nc.sync.dma_start(out=mup, in_=mup_d.ap())
            if nsc > 1:
                dma_xt_sc(1)
            nc.sync.dma_start(out=wo_sb,
                              in_=wo_d.ap().rearrange("(t p) h -> p t h", p=128))
            for sc in range(2, nsc):
                dma_xt_sc(sc)

            for t in ("q", "k", "v"):
                for dt_ in range(2):
                    nc.gpsimd.memset(raw_tiles[(t, dt_)][:, 0:8].bitcast(F32), 0.0)

            th_l = [None] * nchunk
            copy_flip = 0
            for sc in range(nsc):
                base = sc * scs
                # raw projections for this block
                for t in ("q", "k", "v"):
                    w_sb = w_sbs[t]
                    for dt_ in range(2):
                        raw = raw_tiles[(t, dt_)]
                        ps = ppt_b.tile([128, scs], F32, tag="ps", name="psraw")
                        for kk in range(KT):
                            nc.tensor.matmul(
                                ps,
                                w_sb[:, kk, dt_ * 128 : (dt_ + 1) * 128],
                                xt_sb[:, kk, base : base + scs],
                                start=(kk == 0), stop=(kk == KT - 1),
                            )
                        dst = raw[:, 8 + base : 8 + base + scs]
                        if copy_flip % 2 == 0:
                            nc.scalar.activation(out=dst, in_=ps, func=AF.Copy)
                        else:
                            nc.vector.tensor_copy(dst, ps)
                        copy_flip += 1
                # beta matmuls for the chunks of this block (xt only)
                for i in range(sc * cpsc, (sc + 1) * cpsc):
                    ch = slice(i * C, (i + 1) * C)
                    for kk in range(KT):
                        nc.tensor.matmul(
                            psBS[:, 4 * i : 4 * i + 1], xt_sb[:, kk, ch],
                            wb_sb[:, kk, :],
                            start=(kk == 0), stop=(kk == KT - 1),
                        )
                # conv + silu + squared planes
                for t in ("q", "k", "v"):
                    diag = diags[t]
                    for dt_ in range(2):
                        raw = raw_tiles[(t, dt_)]
                        psc = ppt_b.tile([128, scs], F32, tag="ps", name="psconv")
                        for j in (3, 2, 1, 0):
                            sh = 3 - j
                            dslc = diag[:, (j * 2 + dt_) * 128 : (j * 2 + dt_ + 1) * 128]
                            nc.tensor.matmul(
                                psc, dslc,
                                raw[:, 8 + base - sh : 8 + base + scs - sh],
                                start=(j == 3), stop=(j == 0),
                            )
                        plane = planes[(t, dt_)]
                        nc.scalar.activation(
                            out=plane[:, base : base + scs], in_=psc, func=AF.Silu
                        )
                        if t in ("q", "k"):
                            sqv = sq_tiles[(t, dt_)]
                            nc.gpsimd.tensor_mul(
                                sqv[:, base : base + scs],
                                plane[:, base : base + scs],
                                plane[:, base : base + scs],
                            )
                # l2-sum matmuls + tanh(beta) for the chunks of this block
                for i in range(sc * cpsc, (sc + 1) * cpsc):
                    ch = slice(i * C, (i + 1) * C)
                    for col, t in ((1, "q"), (2, "k")):
                        nc.tensor.matmul(
                            psBS[:, 4 * i + col : 4 * i + col + 1],
                            sq_tiles[(t, 0)][:, ch], onescol,
                            start=True, stop=False,
                        )
                        nc.tensor.matmul(
                            psBS[:, 4 * i + col : 4 * i + col + 1],
                            sq_tiles[(t, 1)][:, ch], onescol,
                            start=False, stop=True,
                        )
                    # beta = (1+tanh(x/2))/2 — tanh shares the silu act table
                    th = pbs.tile([128, 1], F32, tag="th", name=f"th_{i}", bufs=nchunk)
                    nc.scalar.activation(
                        out=th, in_=psBS[:, 4 * i : 4 * i + 1], func=AF.Tanh, scale=0.5
                    )
                    th_l[i] = th

            # batched sqrt/recip token scalars (one act-table switch to sqrt)
            for i in range(nchunk):
                sk_s = pbs.tile([128, 1], F32, tag="sk_s", name="sk_s", bufs=4)
                nc.scalar.activation(
                    out=sk_s, in_=psBS[:, 4 * i + 2 : 4 * i + 3],
                    func=AF.Sqrt, bias=eps6,
                )
                ak_t = pbs.tile([128, 1], F32, tag="ak", name="ak", bufs=4)
                nc.vector.reciprocal(out=ak_t, in_=sk_s)
                # bk = beta*ak = (1+th)/2 * ak
                nc.vector.scalar_tensor_tensor(
                    out=toks[i][:, 0:1], in0=th_l[i], scalar=ak_t,
                    in1=ak_t, op0=OP.mult, op1=OP.add,
                )
                nc.vector.tensor_scalar(
                    out=toks[i][:, 0:1], in0=toks[i][:, 0:1], scalar1=0.5,
                    scalar2=None, op0=OP.mult,
                )
                nc.vector.tensor_scalar(
                    out=toks[i][:, 3:4], in0=ak_t, scalar1=-1.0,
                    scalar2=None, op0=OP.mult,
                )
                nc.vector.tensor_mul(toks[i][:, 1:2], toks[i][:, 0:1],
                                     toks[i][:, 3:4])
                sq_s = pbs.tile([128, 1], F32, tag="sq_s", name="sq_s", bufs=4)
                nc.scalar.activation(
                    out=sq_s, in_=psBS[:, 4 * i + 1 : 4 * i + 2],
                    func=AF.Sqrt, bias=eps6,
                )
                nc.vector.reciprocal(out=toks[i][:, 2:3], in_=sq_s)

        # ---------------- phase D: chunked delta rule ----------------
        # PSUM is 8 banks of [128, 512]-f32; pack manually:
        #   bank psS : persistent state, two 256-wide accum groups
        #   bank psc : psKS [:,0:256] + psW [:,256:512]
        #   bank psoD: pso ping-pong by chunk parity
        #   banks op0/op1: o_proj halves
        #   banks pf0/pf1: pass1 f32 scratch, 8 rotating [128,128] slots
        #   bank pbf : bf16 transpose outs — Z(parity) | V | K | OT regions
        dctx = ExitStack()
        pS = dctx.enter_context(tc.tile_pool(name="pS", bufs=3))
        pcs = dctx.enter_context(tc.tile_pool(name="pcs", bufs=4))
        pcm = dctx.enter_context(tc.tile_pool(name="pcm", bufs=2))
        pout = dctx.enter_context(tc.tile_pool(name="pout", bufs=2))
        pds = dctx.enter_context(tc.tile_pool(name="pds", bufs=4))
        ppd = dctx.enter_context(tc.tile_pool(name="ppd", bufs=1, space="PSUM"))

        psS = ppd.tile([128, 512], F32, tag="psS", name="psS")
        psS0 = psS[:, 0:256]
        psS1 = psS[:, 256:512]
        # psum start_tensor_calc zeroes the whole 2KB bank, which would wipe
        # the sibling state half mid-accumulation; instead zero once and
        # accumulate with start=False for all chunks.
        nc.vector.memset(psS, 0.0)
        psc = ppd.tile([128, 512], F32, tag="psc", name="psc")
        psoD = ppd.tile([128, 512], F32, tag="psoD", name="psoD")
        op_t = [ppd.tile([128, 512], F32, tag=f"op{j}", name=f"op{j}") for j in range(2)]
        pf = [ppd.tile([128, 512], F32, tag=f"pf{j}", name=f"pf{j}") for j in range(2)]
        slots32 = [pf[j][:, s * 128 : (s + 1) * 128] for j in range(2) for s in range(4)]
        _slot_ctr = [0]

        def slot32():
            s = slots32[_slot_ctr[0] % 8]
            _slot_ctr[0] += 1
            return s

        pbf = ppd.tile([128, 1024], BF16, tag="pbf", name="pbf")
        psZ_par = [pbf[:, 0:128], pbf[:, 896:1024]]
        psV_r = pbf[:, 128:384]
        psK_r = pbf[:, 384:640]
        psOT_r = pbf[:, 640:896]

        # token-major K and V for all chunks via bulk DMA transposes
        # (DMA engines are otherwise idle in phase D)
        ptm = dctx.enter_context(tc.tile_pool(name="ptm", bufs=1))
        ktok_all = ptm.tile([128, nchunk, 256], BF16, tag="ktok_all")
        vtok_all = ptm.tile([128, nchunk, 256], BF16, tag="vtok_all")
        nc.sync.dma_start_transpose(out=ktok_all[:, :, 0:128], in_=planes[("k", 0)])
        nc.sync.dma_start_transpose(out=ktok_all[:, :, 128:256], in_=planes[("k", 1)])
        nc.sync.dma_start_transpose(out=vtok_all[:, :, 0:128], in_=planes[("v", 0)])
        nc.sync.dma_start_transpose(out=vtok_all[:, :, 128:256], in_=planes[("v", 1)])

        U_l = [None] * nchunk
        Hm_l = [None] * nchunk

        def pass1_steps(i):
            """List of emit-closures, one per ladder rung, for chunk i."""
            ch = slice(i * C, (i + 1) * C)
            k0 = planes[("k", 0)][:, ch]
            k1 = planes[("k", 1)][:, ch]
            q0 = planes[("q", 0)][:, ch]
            q1 = planes[("q", 1)][:, ch]
            tok = toks[i]
            nbk2_t = tok[:, 1:2]
            t_ = {}

            def sb(name, psrc, eng, keep=False):
                dst = pcs.tile([128, 128], BF16, tag=name, name=f"{name}_{i}",
                               bufs=(nchunk if keep else 8))
                if eng == "a":
                    nc.scalar.activation(out=dst, in_=psrc, func=AF.Copy)
                elif eng == "v":
                    nc.vector.tensor_copy(dst, psrc)
                else:
                    nc.gpsimd.tensor_copy(dst, psrc)
                t_[name] = dst
                return dst

            def s_A():
                psA = slot32()
                nc.tensor.matmul(psA, k0, k0, start=True, stop=False)
                nc.tensor.matmul(psA, k1, k1, start=False, stop=True)
                X = pcs.tile([128, 128], BF16, tag="X", name=f"X_{i}", bufs=8)
                nc.vector.scalar_tensor_tensor(
                    out=X, in0=psA, scalar=nbk2_t, in1=mlow, op0=OP.mult, op1=OP.mult
                )
                t_["X"] = X

            def s_Z():
                psZ = psZ_par[i % 2]
                nc.tensor.transpose(psZ, t_["X"], identb)
                sb("Z", psZ, "a")

            def mk_mm(lhs, rhs, name, eng, keep=False):
                def go():
                    psp = slot32()
                    nc.tensor.matmul(psp, t_[lhs], t_[rhs], start=True, stop=True)
                    sb(name, psp, eng, keep=keep)
                return go

            def s_R1():
                psR1 = slot32()
                nc.tensor.matmul(psR1, identb, identb, start=True, stop=False)
                nc.tensor.matmul(psR1, identb, t_["Z"], start=False, stop=False)
                nc.tensor.matmul(psR1, t_["X2"], identb, start=False, stop=False)
                nc.tensor.matmul(psR1, t_["X2"], t_["Z"], start=False, stop=True)
                sb("R1", psR1, "a")

            def s_Q2():
                psQ2 = slot32()
                nc.tensor.matmul(psQ2, identb, identb, start=True, stop=False)
                nc.tensor.matmul(psQ2, identb, t_["X4"], start=False, stop=False)
                nc.tensor.matmul(psQ2, t_["Z4"], t_["X4"], start=False, stop=False)
                nc.tensor.matmul(psQ2, t_["Z8"], t_["X4"], start=False, stop=True)
                sb("Q2", psQ2, "v")

            def s_U():
                # U scaled by bk along its partitions (= contraction tokens):
                # w = U^T (bk*y') = (bk-row-scaled U)^T y'
                psU = slot32()
                nc.tensor.matmul(psU, t_["Q2"], t_["R1"], start=True, stop=True)
                U = pcs.tile([128, 128], BF16, tag="U", name=f"U_{i}", bufs=nchunk)
                nc.vector.tensor_scalar(
                    out=U, in0=psU, scalar1=tok[:, 0:1], scalar2=None, op0=OP.mult
                )
                U_l[i] = U

            def s_H():
                psH = slot32()
                nc.tensor.matmul(psH, k0, q0, start=True, stop=False)
                nc.tensor.matmul(psH, k1, q1, start=False, stop=True)
                Hm = pcs.tile([128, 128], BF16, tag="Hm", name=f"Hm_{i}", bufs=nchunk)
                nc.vector.tensor_mul(Hm, psH, mup)
                Hm_l[i] = Hm

            return [
                s_A, s_Z,
                mk_mm("Z", "X", "X2", "a"),
                mk_mm("X", "Z", "Z2", "v"),
                mk_mm("Z2", "X2", "X4", "v"),
                mk_mm("X2", "Z2", "Z4", "a"),
                mk_mm("X4", "Z4", "Z8", "v"),
                s_R1, s_Q2, s_U, s_H,
            ]

        chain_state = {}

        def emit_chain(i):
            ch = slice(i * C, (i + 1) * C)
            k0 = planes[("k", 0)][:, ch]
            k1 = planes[("k", 1)][:, ch]
            q0 = planes[("q", 0)][:, ch]
            q1 = planes[("q", 1)][:, ch]
            tok = toks[i]
            nbk2_t = tok[:, 1:2]

            if i > 0:
                S_sb = pS.tile([128, 512], BF16, tag="S", name="S_sb")
                nc.vector.tensor_copy(S_sb[:, 0:256], psS0)
                nc.vector.tensor_copy(S_sb[:, 256:512], psS1)
                psKS = psc[:, 0:256]
                nc.tensor.matmul(psKS, k0, S_sb[:, 0:256], start=True, stop=False)
                nc.tensor.matmul(psKS, k1, S_sb[:, 256:512], start=False, stop=True)
                y = pcm.tile([128, 256], BF16, tag="y", name="y", bufs=3)
                nc.vector.scalar_tensor_tensor(
                    out=y, in0=psKS, scalar=tok[:, 3:4], in1=vtok_all[:, i, :],
                    op0=OP.mult, op1=OP.add,
                )
            else:
                S_sb = None
                y = vtok_all[:, i, :]

            # w = U^T y  (U = T^T, so U^T y = T y)
            psW = psc[:, 256:512]
            nc.tensor.matmul(psW, U_l[i], y, start=True, stop=True)
            w = pcm.tile([128, 256], BF16, tag="w", name="w", bufs=3)
            nc.vector.tensor_copy(w, psW)

            # state update S += K^T w (bank pre-zeroed; no start flags)
            nc.tensor.matmul(
                psS0, ktok_all[:, i, 0:128], w,
                start=False, stop=(i == nchunk - 1), skip_group_check=True,
            )
            nc.tensor.matmul(
                psS1, ktok_all[:, i, 128:256], w,
                start=False, stop=(i == nchunk - 1), skip_group_check=True,
            )

            # o = Q S + Hm^T w
            pso = psoD[:, (i % 2) * 256 : (i % 2) * 256 + 256]
            if i > 0:
                nc.tensor.matmul(pso, q0, S_sb[:, 0:256], start=True, stop=False)
                nc.tensor.matmul(pso, q1, S_sb[:, 256:512], start=False, stop=False)
                nc.tensor.matmul(pso, Hm_l[i], w, start=False, stop=True)
            else:
                nc.tensor.matmul(pso, Hm_l[i], w, start=True, stop=True)
            chain_state[i] = pso

        tr_state = {}

        def emit_tr1(i):
            tok = toks[i]
            aq_t = tok[:, 2:3]
            pso = chain_state.pop(i)

            # rms-norm on alpha_q-scaled o, scale folded into the `on` copy
            sums = pds.tile([128, 1], F32, tag="sums", name="sums")
            scratch = pcm.tile([128, 256], F32, tag="scr", name="scratch", bufs=4)
            nc.scalar.activation(out=scratch, in_=pso, func=AF.Square, accum_out=sums)
            aq2 = pds.tile([128, 1], F32, tag="aq2", name="aq2")
            nc.vector.tensor_scalar(
                out=aq2, in0=aq_t, scalar1=aq_t, scalar2=1.0 / D,
                op0=OP.mult, op1=OP.mult,
            )
            rstd_t = pds.tile([128, 1], F32, tag="rstd_t", name="rstd_t")
            nc.scalar.activation(
                out=rstd_t, in_=sums, func=AF.Sqrt, scale=aq2[:, 0:1], bias=eps5
            )
            rstd = pds.tile([128, 1], F32, tag="rstd", name="rstd")
            nc.vector.reciprocal(out=rstd, in_=rstd_t)
            saq = pds.tile([128, 1], F32, tag="saq", name="saq")
            nc.vector.tensor_mul(saq, rstd, aq_t)
            on = pcm.tile([128, 256], BF16, tag="on", name="on", bufs=4)
            nc.scalar.activation(
                out=on, in_=pso, func=AF.Copy, scale=saq[:, 0:1]
            )
            tr_state[i] = on

        def emit_tr2(i):
            on = tr_state.pop(i)
            psOT = psOT_r
            nc.tensor.transpose(psOT[:, 0:128], on[:, 0:128], identb)
            nc.tensor.transpose(psOT[:, 128:256], on[:, 128:256], identb)
            ot = pcm.tile([128, 256], BF16, tag="ot", name="ot", bufs=4)
            nc.scalar.activation(out=ot, in_=psOT, func=AF.Copy)
            tr_state[("ot", i)] = ot

        def emit_tr3(i):
            ch = slice(i * C, (i + 1) * C)
            ot = tr_state.pop(("ot", i))
            outbuf = pout.tile([128, HID], F32, tag="outbuf", name="outbuf")
            for hc in range(2):
                psop = op_t[hc]
                nc.tensor.matmul(
                    psop, ot[:, 0:128], wo_sb[:, 0, hc * 512 : (hc + 1) * 512],
                    start=True, stop=False,
                )
                nc.tensor.matmul(
                    psop, ot[:, 128:256], wo_sb[:, 1, hc * 512 : (hc + 1) * 512],
                    start=False, stop=True,
                )
                if hc == 0:
                    nc.scalar.activation(out=outbuf[:, 0:512], in_=psop, func=AF.Copy)
                else:
                    nc.scalar.activation(out=outbuf[:, 512:1024], in_=psop,
                                         func=AF.Copy)
            nc.sync.dma_start(out=out_d.ap()[ch, :], in_=outbuf)

        # diagonal (skewed) schedule: chunk i's stage r emits at clock i + r,
        # so consecutive PE instructions belong to different chunks and the
        # in-order PE queue never waits on a just-issued copy.
        stages = {}
        for i in range(nchunk):
            steps = pass1_steps(i)
            steps.append(lambda i=i: emit_chain(i))
            steps.append(lambda i=i: emit_tr1(i))
            steps.append(lambda i=i: emit_tr2(i))
            steps.append(lambda i=i: emit_tr3(i))
            stages[i] = steps
        M = len(stages[0])
        for c in range(nchunk + M - 1):
            for i in range(max(0, c - M + 1), min(c, nchunk - 1) + 1):
                stages[i][c - i]()
        dctx.close()

    nc.compile()
    return nc


def make_host_inputs(inputs, nchunk=S_FULL // C):
    """Shard + preprocess full inputs into per-core in_maps."""
    import ml_dtypes

    bf16 = ml_dtypes.bfloat16
    S = nchunk * C
    hs = np.ascontiguousarray(np.asarray(inputs["hidden_states"])[:, :S, :]).astype(
        np.float32
    )
    Wq, Wk, Wv = (np.asarray(inputs[k], np.float32) for k in ("Wq", "Wk", "Wv"))
    Wb = np.asarray(inputs["Wb"], np.float32)
    Wo = np.asarray(inputs["Wo"], np.float32)
    nw = np.asarray(inputs["norm_w"], np.float32)
    convs = {
        k: np.asarray(inputs[k], np.float32) for k in ("conv_q", "conv_k", "conv_v")
    }

    identb = np.eye(128, dtype=bf16)
    onescol = np.ones((128, 1), bf16)
    mlow = np.tril(np.ones((128, 128), np.float32), -1)
    mup = np.triu(np.ones((128, 128), np.float32), 0)

    def diag_pack(cw):
        # cw: [256, 4] tap weights for this head -> [128, 8*128]
        out = np.zeros((128, 8 * 128), np.float32)
        for j in range(4):
            for dt_ in range(2):
                blk = np.diag(cw[dt_ * 128 : (dt_ + 1) * 128, j])
                out[:, (j * 2 + dt_) * 128 : (j * 2 + dt_ + 1) * 128] = blk
        return out

    in_maps = []
    for core in range(8):
        b, h = core // 4, core % 4
        hsel = slice(h * D, (h + 1) * D)
        in_maps.append(
            {
                "xt": np.ascontiguousarray(hs[b].T).astype(bf16),
                "wq": np.ascontiguousarray(Wq[:, hsel]).astype(bf16),
                "wk": np.ascontiguousarray(Wk[:, hsel]).astype(bf16),
                "wv": np.ascontiguousarray(Wv[:, hsel]).astype(bf16),
                "wb": np.ascontiguousarray(Wb[:, h : h + 1]).astype(bf16),
                "wo": np.ascontiguousarray((nw[:, None] * Wo[hsel, :]).astype(bf16)),
                "cdq": diag_pack(convs["conv_q"][hsel]),
                "cdk": diag_pack(convs["conv_k"][hsel]),
                "cdv": diag_pack(convs["conv_v"][hsel]),
                "identb": identb,
                "onescol": onescol,
                "mlow": mlow,
                "mup": mup,
            }
        )
    return in_maps


_NC_CACHE = {}


def _get_nc(nchunk):
    if nchunk not in _NC_CACHE:
        _NC_CACHE[nchunk] = build_nc(nchunk)
    return _NC_CACHE[nchunk]


def kernel(**inputs) -> np.ndarray:
    nchunk = S_FULL // C
    nc = _get_nc(nchunk)
    in_maps = make_host_inputs(inputs, nchunk)
    res = run_bass_kernel_spmd(nc, in_maps, core_ids=list(range(8)))
    S = nchunk * C
    out = np.zeros((B, S, HID), np.float32)
    for core in range(8):
        out[core // 4] += res.results[core]["out"]
    return out
